# revision 13
# baseline (speedup 1.0000x reference)
"""Bass/Trainium2 kernel for nn_Attn (Bahdanau 'general' attention scoring).

Reference math:
    energies = einsum('sd,hd,h->s', enc, W, hidden) + b.hidden
    out      = softmax(energies)[None, None, :]

Factorization:
    v = W^T @ hidden (200-dim), energies = enc @ v (+ const; softmax cancels
    the constant b.hidden term, so b is dropped).

Distribution (8 NeuronCores, one TRN2 chip) — sequence sharding with a
replicated W. Profile-driven rationale: the per-execution collective
machinery on this runtime is a fixed ~65-78us chain (entry barrier ~40us
after the last core's first collective doorbell + ~11us first-collective
setup + warm-op), so ALL heavy local work is hidden under that window
and the only data collective is made as small as possible:
  - Every core loads the FULL W and computes v = W^T @ hidden locally
    (no v collective): elementwise mult+reduce on the DVE over
    [128, d, 64] chunks (small-first chunk sizes so the DVE starts as
    soon as the first W chunk lands), partition-collapsed+broadcast by
    one ones-matmul on the PE. GpSimd is NOT used for tensor work: DVE
    and GpSimd share SBUF ports, so concurrent elementwise work caps at
    the same ~116G elem/s as DVE alone.
  - Core i owns seq slice [4096*i, 4096*(i+1)): energies e = enc_i @ v
    on the DVE, laid [128, 32] (s_local = p*32 + f).
  - Local softmax, all pre-collective: m_c = core max via DVE reduce +
    gpsimd.partition_all_reduce(max); q = exp(e - m_c) with per-row
    accumulation, summed across partitions by partition_all_reduce(add).
  - ONE tiny AllGather of (m_c, s_c) pairs (16B total payload) replaces
    a 128KB energy AllReduce: global M = max_c m_c,
    S = sum_c s_c*exp(m_c-M), out_i = q * alpha with
    alpha = exp(m_c - M)/S. Each core writes only its 4096-slice; the
    host concatenates the 8 shards.
  - A dependency-free warm-up AllGather rings the collective doorbell
    right after the fixed engine preamble so the barrier+setup runs
    concurrently with the DMAs/compute. Its sink DMA is pinned to the
    end of the schedule via tile_wait_until so the Tile scheduler
    cannot place it ahead of real work on the same queue (its wait on
    the warm collective would stall that queue for ~50us).
"""

import numpy as np

N_CORES = 8
SEQ = 32768
D = 200
H = 8192
P = 128
KCH = H // P            # 64 h-chunks
S_LOCAL = SEQ // N_CORES    # 4096 positions per core
FSH = S_LOCAL // P      # 32 free positions per partition
# v-compute d-chunks, small-first so the DVE starts on the first W DMA.
# Energies are computed per-chunk too (e += enc[:,:,c] @ v[c]) so the
# energy multiplies interleave into the DVE's W-DMA wait bubbles.
W_CHUNKS = [(0, 24), (24, 56), (56, 104), (104, 152), (152, 200)]


def build_kernel():
    import concourse.bacc as bacc
    import concourse.bass_isa as bass_isa
    import concourse.mybir as mybir
    import concourse.tile as tile

    fp32 = mybir.dt.float32
    nc = bacc.Bacc(
        "TRN2",
        target_bir_lowering=False,
        debug=False,
        num_devices=N_CORES,
    )

    # Host-prepacked layouts (see shard_inputs):
    #   encP [128, 32*200]: [p, f, d] with s_local = p*32 + f
    #   wP   [128, 200*64]: [p, d, k] with h = k*128 + p  (k innermost)
    #   hidP [128, 64]:     [p, k]    with h = k*128 + p
    encP = nc.dram_tensor("encP", [P, FSH * D], fp32, kind="ExternalInput")
    wP = nc.dram_tensor("wP", [P, D * KCH], fp32, kind="ExternalInput")
    hidP = nc.dram_tensor("hidP", [P, KCH], fp32, kind="ExternalInput")
    out = nc.dram_tensor("out", [S_LOCAL], fp32, kind="ExternalOutput")
    # Sink for the warm-up collective (kept live so it isn't DCE'd).
    warm_out = nc.dram_tensor("warm_out", [2, 4], fp32,
                              kind="ExternalOutput")

    rg8 = [list(range(N_CORES))]

    with tile.TileContext(nc) as tc:
        with (
            tc.tile_pool(name="const", bufs=1) as constp,
            tc.tile_pool(name="sb", bufs=1) as sb,
            tc.tile_pool(name="ps", bufs=1, space="PSUM") as ps,
            tc.tile_pool(name="dram", bufs=1, space="DRAM") as dram,
        ):
            # ---- warm-up collective, FIRST and with NO data dependencies:
            # rings the runtime's collective doorbell immediately after the
            # fixed engine preamble so the ~40us entry barrier + ~11us
            # first-collective setup run while the DMAs/compute proceed.
            warm_b = nc.inline_tensor(np.zeros((1, 4), np.float32),
                                      name="warm_src")
            warm_g = dram.tile([2, 4], fp32)
            nc.gpsimd.collective_compute(
                "AllGather",
                mybir.AluOpType.bypass,
                replica_groups=[[2 * i, 2 * i + 1] for i in range(N_CORES // 2)],
                ins=[warm_b.ap().opt()],
                outs=[warm_g[:].opt()],
            )

            ones = constp.tile([P, P], fp32)
            nc.vector.memset(ones[:], 1.0)

            # ---- loads (hid + W first: they gate the v chain). enc goes
            # through the Scalar engine's DMA queue so its issue does not
            # serialize behind the W chunks on the Sync queue. ----
            h_sb = sb.tile([P, KCH], fp32)
            nc.sync.dma_start(h_sb[:], hidP.ap())
            w_sb = sb.tile([P, D * KCH], fp32)
            for d0, d1 in W_CHUNKS:
                sl = slice(d0 * KCH, d1 * KCH)
                nc.sync.dma_start(w_sb[:, sl], wP.ap()[:, sl])
            enc_sb = sb.tile([P, FSH * D], fp32)
            half = FSH * D // 2
            nc.scalar.dma_start(enc_sb[:, 0:half], encP.ap()[:, 0:half])
            nc.scalar.dma_start(enc_sb[:, half:], encP.ap()[:, half:])

            # ---- v = W^T @ hidden and e = enc @ v, chunked over d and
            # interleaved on the DVE: for each W chunk, DVE computes the
            # per-partition v partials (mult+reduce over [128, dn, 64]),
            # the PE collapses partitions + broadcasts (ones-matmul), and
            # the chunk's energy contribution e += enc[:,:,c] @ v[c] slots
            # into the DVE stream while later W chunks are still in DMA.
            w3 = w_sb[:].rearrange("p (d k) -> p d k", d=D)
            enc3 = enc_sb[:].rearrange("p (f d) -> p f d", d=D)
            vpart = sb.tile([P, D], fp32)
            v_sb = sb.tile([P, D], fp32)
            e_sb = sb.tile([P, FSH], fp32)

            def v_chunk(d0, d1):
                dn = d1 - d0
                prod = sb.tile([P, dn * KCH], fp32, tag="prod", bufs=2)
                h_b = (
                    h_sb[:]
                    .rearrange("p k -> p () k")
                    .broadcast_to([P, dn, KCH])
                )
                nc.vector.tensor_tensor(
                    out=prod[:].rearrange("p (d k) -> p d k", d=dn),
                    in0=w3[:, d0:d1, :],
                    in1=h_b,
                    op=mybir.AluOpType.mult,
                )
                nc.vector.reduce_sum(
                    vpart[:, d0:d1],
                    prod[:].rearrange("p (d k) -> p d k", d=dn),
                    axis=mybir.AxisListType.X,
                )
                v_ps = ps.tile([P, dn], fp32, tag="vps", bufs=2)
                nc.tensor.matmul(
                    v_ps[:], lhsT=ones[:], rhs=vpart[:, d0:d1],
                    start=True, stop=True,
                )
                nc.scalar.copy(v_sb[:, d0:d1], v_ps[:])

            def e_chunk(ci):
                d0, d1 = W_CHUNKS[ci]
                dn = d1 - d0
                eprod = sb.tile([P, FSH * dn], fp32, tag="eprod", bufs=2)
                v_b = (
                    v_sb[:, d0:d1]
                    .rearrange("p d -> p () d")
                    .broadcast_to([P, FSH, dn])
                )
                nc.vector.tensor_tensor(
                    out=eprod[:].rearrange("p (f d) -> p f d", d=dn),
                    in0=enc3[:, :, d0:d1],
                    in1=v_b,
                    op=mybir.AluOpType.mult,
                )
                if ci == 0:
                    nc.vector.reduce_sum(
                        e_sb[:],
                        eprod[:].rearrange("p (f d) -> p f d", d=dn),
                        axis=mybir.AxisListType.X,
                    )
                else:
                    e_tmp = sb.tile([P, FSH], fp32, tag="etmp", bufs=2)
                    nc.vector.reduce_sum(
                        e_tmp[:],
                        eprod[:].rearrange("p (f d) -> p f d", d=dn),
                        axis=mybir.AxisListType.X,
                    )
                    nc.vector.tensor_tensor(
                        e_sb[:], e_sb[:], e_tmp[:], op=mybir.AluOpType.add
                    )

            v_chunk(*W_CHUNKS[0])
            v_chunk(*W_CHUNKS[1])
            v_chunk(*W_CHUNKS[2])
            e_chunk(0)
            v_chunk(*W_CHUNKS[3])
            e_chunk(1)
            v_chunk(*W_CHUNKS[4])
            e_chunk(2)
            e_chunk(3)
            e_chunk(4)

            # ---- local softmax prep (all before the stats collective):
            # m_c = max(e) over the whole core, q = exp(e - m_c),
            # s_c = sum(q). PAR writes land in adjacent columns so the
            # bounce DMA reads (m_c, s_c) directly with no copies. ----
            m_p = sb.tile([P, 1], fp32)
            nc.vector.reduce_max(m_p[:], e_sb[:], axis=mybir.AxisListType.X)
            stat2 = sb.tile([P, 2], fp32)
            nc.gpsimd.partition_all_reduce(
                stat2[:, 0:1], m_p[:], channels=P,
                reduce_op=bass_isa.ReduceOp.max,
            )
            neg_mc = sb.tile([P, 1], fp32)
            nc.vector.tensor_scalar_mul(neg_mc[:], stat2[:, 0:1], -1.0)
            q = sb.tile([P, FSH], fp32)
            s_p = sb.tile([P, 1], fp32)
            nc.scalar.activation(
                q[:], e_sb[:], mybir.ActivationFunctionType.Exp,
                bias=neg_mc[:], scale=1.0, accum_out=s_p[:],
            )
            nc.gpsimd.partition_all_reduce(
                stat2[:, 1:2], s_p[:], channels=P,
                reduce_op=bass_isa.ReduceOp.add,
            )

            # ---- tiny stats AllGather: (m_c, s_c) pairs, 16B payload ----
            bounce = dram.tile([1, 2], fp32)
            statsg = dram.tile([1, 2 * N_CORES], fp32, addr_space="Shared")
            nc.sync.dma_start(bounce[:], stat2[0:1, :])
            nc.gpsimd.collective_compute(
                "AllGather",
                mybir.AluOpType.bypass,
                replica_groups=rg8,
                ins=[bounce[:].opt()],
                outs=[statsg[:].opt()],
            )
            sg = sb.tile([1, 2 * N_CORES], fp32)
            nc.sync.dma_start(sg[:], statsg[:])

            # ---- global combine: M = max_c m_c, S = sum_c s_c*exp(m_c-M),
            # alpha = exp(m_c - M) / S, out = q * alpha ----
            sg2 = sg[:].rearrange("a (r two) -> a r two", two=2)
            m_view = sg2[:, :, 0]
            s_view = sg2[:, :, 1]
            negM = sb.tile([1, 1], fp32)
            nc.vector.reduce_max(negM[:], m_view, axis=mybir.AxisListType.X,
                                 negate=True)
            wexp = sb.tile([1, N_CORES], fp32)
            nc.scalar.activation(
                wexp[:], m_view, mybir.ActivationFunctionType.Exp,
                bias=negM[:], scale=1.0,
            )
            alpha_e = sb.tile([1, 1], fp32)
            nc.scalar.activation(
                alpha_e[:], stat2[0:1, 0:1], mybir.ActivationFunctionType.Exp,
                bias=negM[:], scale=1.0,
            )
            sw = sb.tile([1, N_CORES], fp32)
            nc.vector.tensor_tensor(sw[:], wexp[:], s_view,
                                    op=mybir.AluOpType.mult)
            S_sum = sb.tile([1, 1], fp32)
            nc.vector.reduce_sum(S_sum[:], sw[:], axis=mybir.AxisListType.X)
            rS = sb.tile([1, 1], fp32)
            nc.vector.reciprocal(rS[:], S_sum[:])
            alpha = sb.tile([1, 1], fp32)
            nc.vector.tensor_tensor(alpha[:], alpha_e[:], rS[:],
                                    op=mybir.AluOpType.mult)
            alpha_bc = sb.tile([P, 1], fp32)
            nc.gpsimd.partition_broadcast(alpha_bc[:], alpha[:])
            o_sb = sb.tile([P, FSH], fp32)
            nc.vector.tensor_scalar_mul(o_sb[:], q[:], alpha_bc[:])
            nc.sync.dma_start(out.ap().rearrange("(p f) -> p f", p=P), o_sb[:])

            # Keep the warm-up collective live. tile_wait_until pins it to
            # the end of the Tile scheduler's timeline so its wait on the
            # warm AllGather never stalls real work queued after it.
            with tc.tile_wait_until(1.0):
                nc.scalar.dma_start(warm_out.ap(), warm_g[:])

    nc.compile()
    return nc


def shard_inputs(hidden, encoder_outputs, W, b):
    hidden = np.asarray(hidden, dtype=np.float32)
    enc = np.asarray(encoder_outputs, dtype=np.float32)
    W = np.asarray(W, dtype=np.float32)
    # wP: [p, d, k] with h = k*128 + p
    wP = np.ascontiguousarray(
        W.reshape(KCH, P, D).transpose(1, 2, 0)
    ).reshape(P, D * KCH)
    hidP = np.ascontiguousarray(hidden.reshape(KCH, P).T)  # [p, k]
    in_maps = []
    for i in range(N_CORES):
        shard = enc[i * S_LOCAL:(i + 1) * S_LOCAL]          # [4096, 200]
        encP_i = np.ascontiguousarray(shard).reshape(P, FSH * D)
        in_maps.append({"encP": encP_i, "wP": wP, "hidP": hidP})
    return in_maps


_NC_CACHE = {}


def _get_nc():
    if "nc" not in _NC_CACHE:
        _NC_CACHE["nc"] = build_kernel()
    return _NC_CACHE["nc"]


def kernel(hidden, encoder_outputs, W, b):
    from concourse import bass_utils

    nc = _get_nc()
    in_maps = shard_inputs(hidden, encoder_outputs, W, b)
    res = bass_utils.run_bass_kernel_spmd(
        nc, in_maps, core_ids=list(range(N_CORES))
    )
    out = np.concatenate(
        [np.asarray(res.results[c]["out"], dtype=np.float32)
         for c in range(N_CORES)]
    )
    return out.reshape(1, 1, SEQ)


# revision 14
# speedup vs baseline: 1.2866x; 1.2866x over previous
"""Bass/Trainium2 kernel for nn_Attn (Bahdanau 'general' attention scoring).

Reference math:
    energies = einsum('sd,hd,h->s', enc, W, hidden) + b.hidden
    out      = softmax(energies)[None, None, :]

Factorization:
    v = W^T @ hidden (200-dim), energies = enc @ v (+ const; softmax cancels
    the constant b.hidden term, so b is dropped).

Distribution (8 NeuronCores, one TRN2 chip) — d-sharding: core i owns
d-slice [25*i, 25*(i+1)) of the contraction dim:
    W slice  [8192, 25]  -> v_i = W_i^T @ hidden (exact, local, no comm)
    enc slice [32768, 25] -> partial energies e_i[s] = enc[s, d_i] . v_i
for ALL 32768 positions, laid out [128, 256] (s = p*256 + f), then ONE
AllReduce(add) over the 128KB partials. Rationale from warmed profiles:
  - The collective entry barrier + first-collective setup complete by
    ~50us (global) when a dependency-free warm-up AllGather rings the
    doorbell right after the engine preamble; the AllReduce's own
    doorbell (~37us local + up to ~20us core start skew) is what gates
    it, so the cheap d-sharded local compute (only ~18us of DVE work
    vs ~42us for a replicated-W sequence-sharded variant) keeps the
    whole pre-collective phase inside the skew+bootstrap window.
  - The post-AllReduce softmax uses gpsimd.partition_all_reduce for the
    cross-partition max/sum instead of PE-transpose chains: the tail is
    ~6 ops instead of 11, cutting ~5us of serial semaphore-latency.
  - Every core computes the identical softmax and writes the full
    output; the host takes core 0's copy.
  - The warm-up collective's sink DMA is pinned to the end of the
    schedule via tile_wait_until so the Tile scheduler cannot place it
    ahead of real work on the same engine queue (its wait on the warm
    AllGather would stall that queue for ~50us).
"""

import numpy as np

N_CORES = 8
SEQ = 32768
D = 200
H = 8192
DSH = D // N_CORES      # 25
P = 128
F = SEQ // P            # 256
KCH = H // P            # 64
NCH = 4                 # enc DMA / DVE chunks along F
FC = F // NCH           # 64


def build_kernel():
    import concourse.bacc as bacc
    import concourse.bass_isa as bass_isa
    import concourse.mybir as mybir
    import concourse.tile as tile

    fp32 = mybir.dt.float32
    nc = bacc.Bacc(
        "TRN2",
        target_bir_lowering=False,
        debug=False,
        num_devices=N_CORES,
    )

    # Host-prepacked layouts (see shard_inputs):
    #   encP [128, 256*25]: [p, f, d] with global s = p*256 + f
    #   wP   [128, 25*64]:  [p, d, k] with h = k*128 + p  (d-major!)
    #   hidP [128, 64]:     [p, k]    with h = k*128 + p
    encP = nc.dram_tensor("encP", [P, F * DSH], fp32, kind="ExternalInput")
    wP = nc.dram_tensor("wP", [P, DSH * KCH], fp32, kind="ExternalInput")
    hidP = nc.dram_tensor("hidP", [P, KCH], fp32, kind="ExternalInput")
    out = nc.dram_tensor("out", [SEQ], fp32, kind="ExternalOutput")
    # Sink for the warm-up collective (kept live so it isn't DCE'd).
    warm_out = nc.dram_tensor("warm_out", [2, 4], fp32,
                              kind="ExternalOutput")

    rg = [list(range(N_CORES))]

    with tile.TileContext(nc) as tc:
        with (
            tc.tile_pool(name="const", bufs=1) as constp,
            tc.tile_pool(name="sb", bufs=1) as sb,
            tc.tile_pool(name="ps", bufs=1, space="PSUM") as ps,
            tc.tile_pool(name="dram", bufs=1, space="DRAM") as dram,
        ):
            # ---- warm-up collective, FIRST and with NO data dependencies:
            # rings the runtime's collective doorbell immediately after the
            # fixed engine preamble so the entry barrier + first-collective
            # setup run while the DMAs/DVE work proceed. Pair groups: the
            # pairwise mesh completes faster than the 8-core one.
            warm_b = nc.inline_tensor(np.zeros((1, 4), np.float32),
                                      name="warm_src")
            warm_g = dram.tile([2, 4], fp32)
            nc.gpsimd.collective_compute(
                "AllGather",
                mybir.AluOpType.bypass,
                replica_groups=[[2 * i, 2 * i + 1] for i in range(N_CORES // 2)],
                ins=[warm_b.ap().opt()],
                outs=[warm_g[:].opt()],
            )

            ones = constp.tile([P, P], fp32)
            nc.vector.memset(ones[:], 1.0)

            # ---- loads (w + hid first: they gate the v chain) ----
            h_sb = sb.tile([P, KCH], fp32)
            nc.sync.dma_start(h_sb[:], hidP.ap())
            w_sb = sb.tile([P, DSH * KCH], fp32)
            nc.sync.dma_start(w_sb[:], wP.ap())
            enc_sb = sb.tile([P, F * DSH], fp32)
            for c in range(NCH):
                sl = slice(c * FC * DSH, (c + 1) * FC * DSH)
                nc.sync.dma_start(enc_sb[:, sl], encP.ap()[:, sl])

            # ---- v_i = W_i^T @ hidden (DVE mult + unit-stride reduce) ----
            prod_w = sb.tile([P, DSH * KCH], fp32)
            h_b = (
                h_sb[:]
                .rearrange("p k -> p () k")
                .broadcast_to([P, DSH, KCH])
            )
            nc.vector.tensor_tensor(
                out=prod_w[:].rearrange("p (d k) -> p d k", d=DSH),
                in0=w_sb[:].rearrange("p (d k) -> p d k", d=DSH),
                in1=h_b,
                op=mybir.AluOpType.mult,
            )
            vtmp = sb.tile([P, DSH], fp32)
            nc.vector.reduce_sum(
                vtmp[:],
                prod_w[:].rearrange("p (d k) -> p d k", d=DSH),
                axis=mybir.AxisListType.X,
            )
            # one matmul: column-sums broadcast to every partition
            v_bc_ps = ps.tile([P, DSH], fp32, tag="vbc")
            nc.tensor.matmul(
                v_bc_ps[:], lhsT=ones[:], rhs=vtmp[:], start=True, stop=True
            )
            v_bc = sb.tile([P, DSH], fp32)
            nc.scalar.copy(v_bc[:], v_bc_ps[:])

            # ---- partial energies e_i[p, f] = sum_d enc[p, f, d] * v[d] ----
            e_part = sb.tile([P, F], fp32)
            for c in range(NCH):
                sl3 = enc_sb[:].rearrange("p (f d) -> p f d", d=DSH)[
                    :, c * FC : (c + 1) * FC, :
                ]
                eprod = sb.tile([P, FC * DSH], fp32, tag="eprod", bufs=2)
                v_b = (
                    v_bc[:]
                    .rearrange("p d -> p () d")
                    .broadcast_to([P, FC, DSH])
                )
                nc.vector.tensor_tensor(
                    out=eprod[:].rearrange("p (f d) -> p f d", d=DSH),
                    in0=sl3,
                    in1=v_b,
                    op=mybir.AluOpType.mult,
                )
                nc.vector.reduce_sum(
                    e_part[:, c * FC : (c + 1) * FC],
                    eprod[:].rearrange("p (f d) -> p f d", d=DSH),
                    axis=mybir.AxisListType.X,
                )

            # ---- AllReduce the partial energies ----
            bounce = dram.tile([P, F], fp32)
            esum = dram.tile([P, F], fp32, addr_space="Shared")
            nc.sync.dma_start(bounce[:, 0 : F // 2], e_part[:, 0 : F // 2])
            nc.sync.dma_start(bounce[:, F // 2 : F], e_part[:, F // 2 : F])
            nc.gpsimd.collective_compute(
                "AllReduce",
                mybir.AluOpType.add,
                replica_groups=rg,
                ins=[bounce[:].opt()],
                outs=[esum[:].opt()],
            )
            e_sb = sb.tile([P, F], fp32)
            nc.sync.dma_start(e_sb[:], esum[:])

            # ---- replicated softmax over [128, 256] via partition_all_reduce
            # (short serial chain, no PE transposes) ----
            m_p = sb.tile([P, 1], fp32)
            nc.vector.reduce_max(m_p[:], e_sb[:], axis=mybir.AxisListType.X)
            M_bc = sb.tile([P, 1], fp32)
            nc.gpsimd.partition_all_reduce(
                M_bc[:], m_p[:], channels=P, reduce_op=bass_isa.ReduceOp.max
            )
            negM = sb.tile([P, 1], fp32)
            nc.vector.tensor_scalar_mul(negM[:], M_bc[:], -1.0)
            q = sb.tile([P, F], fp32)
            s_p = sb.tile([P, 1], fp32)
            nc.scalar.activation(
                q[:], e_sb[:], mybir.ActivationFunctionType.Exp,
                bias=negM[:], scale=1.0, accum_out=s_p[:],
            )
            S_bc = sb.tile([P, 1], fp32)
            nc.gpsimd.partition_all_reduce(
                S_bc[:], s_p[:], channels=P, reduce_op=bass_isa.ReduceOp.add
            )
            rS = sb.tile([P, 1], fp32)
            nc.vector.reciprocal(rS[:], S_bc[:])
            o_sb = sb.tile([P, F], fp32)
            nc.vector.tensor_scalar_mul(o_sb[:], q[:], rS[:])
            nc.sync.dma_start(out.ap().rearrange("(p f) -> p f", p=P), o_sb[:])

            # Keep the warm-up collective live. tile_wait_until pins it to
            # the end of the Tile scheduler's timeline so its wait on the
            # warm AllGather never stalls real work queued after it.
            with tc.tile_wait_until(1.0):
                nc.scalar.dma_start(warm_out.ap(), warm_g[:])

    nc.compile()
    return nc


def shard_inputs(hidden, encoder_outputs, W, b):
    hidden = np.asarray(hidden, dtype=np.float32)
    enc = np.asarray(encoder_outputs, dtype=np.float32)
    W = np.asarray(W, dtype=np.float32)
    enc3 = enc.reshape(P, F, D)          # s = p*F + f
    w3 = W.reshape(KCH, P, D)            # h = k*P + p
    hidP = np.ascontiguousarray(hidden.reshape(KCH, P).T)  # [p, k]
    in_maps = []
    for i in range(N_CORES):
        sl = slice(i * DSH, (i + 1) * DSH)
        encP_i = np.ascontiguousarray(enc3[:, :, sl]).reshape(P, F * DSH)
        wP_i = np.ascontiguousarray(
            w3[:, :, sl].transpose(1, 2, 0)       # [p, d, k]
        ).reshape(P, DSH * KCH)
        in_maps.append({"encP": encP_i, "wP": wP_i, "hidP": hidP})
    return in_maps


_NC_CACHE = {}


def _get_nc():
    if "nc" not in _NC_CACHE:
        _NC_CACHE["nc"] = build_kernel()
    return _NC_CACHE["nc"]


def kernel(hidden, encoder_outputs, W, b):
    from concourse import bass_utils

    nc = _get_nc()
    in_maps = shard_inputs(hidden, encoder_outputs, W, b)
    res = bass_utils.run_bass_kernel_spmd(
        nc, in_maps, core_ids=list(range(N_CORES))
    )
    out = np.asarray(res.results[0]["out"], dtype=np.float32)
    return out.reshape(1, 1, SEQ)


# revision 16
# speedup vs baseline: 1.3682x; 1.0634x over previous
"""Bass/Trainium2 kernel for nn_Attn (Bahdanau 'general' attention scoring).

Reference math:
    energies = einsum('sd,hd,h->s', enc, W, hidden) + b.hidden
    out      = softmax(energies)[None, None, :]

Factorization:
    v = W^T @ hidden (200-dim), energies = enc @ v (+ const; softmax cancels
    the constant b.hidden term, so b is dropped).

Distribution (8 NeuronCores, one TRN2 chip) — d-sharding: core i owns
d-slice [25*i, 25*(i+1)) of the contraction dim:
    W slice  [8192, 25]  -> v_i = W_i^T @ hidden (exact, local, no comm)
    enc slice [32768, 25] -> partial energies e_i[s] = enc[s, d_i] . v_i
for ALL 32768 positions, laid out [128, 256] (s = p*256 + f), then ONE
AllReduce(add) over the 128KB partials. Rationale from warmed profiles:
  - The collective entry barrier + first-collective setup complete by
    ~50us (global) when a dependency-free warm-up AllGather rings the
    doorbell right after the engine preamble; the AllReduce's own
    doorbell (~37us local + up to ~20us core start skew) is what gates
    it, so the cheap d-sharded local compute (only ~18us of DVE work
    vs ~42us for a replicated-W sequence-sharded variant) keeps the
    whole pre-collective phase inside the skew+bootstrap window.
  - The post-AllReduce softmax uses gpsimd.partition_all_reduce for the
    cross-partition max/sum instead of PE-transpose chains: the tail is
    ~6 ops instead of 11, cutting ~5us of serial semaphore-latency.
  - Every core computes the identical softmax and writes the full
    output; the host takes core 0's copy.
  - The warm-up collective's sink DMA is pinned to the end of the
    schedule via tile_wait_until so the Tile scheduler cannot place it
    ahead of real work on the same engine queue (its wait on the warm
    AllGather would stall that queue for ~50us).
"""

import numpy as np

N_CORES = 8
SEQ = 32768
D = 200
H = 8192
DSH = D // N_CORES      # 25
P = 128
F = SEQ // P            # 256
KCH = H // P            # 64
NCH = 4                 # enc DMA / DVE chunks along F
FC = F // NCH           # 64


def build_kernel():
    import concourse.bacc as bacc
    import concourse.bass_isa as bass_isa
    import concourse.mybir as mybir
    import concourse.tile as tile

    fp32 = mybir.dt.float32
    nc = bacc.Bacc(
        "TRN2",
        target_bir_lowering=False,
        debug=False,
        num_devices=N_CORES,
    )

    # Host-prepacked layouts (see shard_inputs):
    #   encP [128, 256*25]: [p, f, d] with global s = p*256 + f
    #   wP   [128, 25*64]:  [p, d, k] with h = k*128 + p  (d-major!)
    #   hidP [128, 64]:     [p, k]    with h = k*128 + p
    encP = nc.dram_tensor("encP", [P, F * DSH], fp32, kind="ExternalInput")
    wP = nc.dram_tensor("wP", [P, DSH * KCH], fp32, kind="ExternalInput")
    hidP = nc.dram_tensor("hidP", [P, KCH], fp32, kind="ExternalInput")
    out = nc.dram_tensor("out", [SEQ], fp32, kind="ExternalOutput")
    # Sink for the warm-up collective (kept live so it isn't DCE'd).
    warm_out = nc.dram_tensor("warm_out", [2, 4], fp32,
                              kind="ExternalOutput")

    rg = [list(range(N_CORES))]

    with tile.TileContext(nc) as tc:
        with (
            tc.tile_pool(name="const", bufs=1) as constp,
            tc.tile_pool(name="sb", bufs=1) as sb,
            tc.tile_pool(name="ps", bufs=1, space="PSUM") as ps,
            tc.tile_pool(name="dram", bufs=1, space="DRAM") as dram,
        ):
            # ---- warm-up collective, FIRST and with NO data dependencies:
            # rings the runtime's collective doorbell immediately after the
            # fixed engine preamble so the entry barrier + first-collective
            # setup run while the DMAs/DVE work proceed. Pair groups: the
            # pairwise mesh completes faster than the 8-core one.
            warm_b = nc.inline_tensor(np.zeros((1, 4), np.float32),
                                      name="warm_src")
            warm_g = dram.tile([2, 4], fp32)
            nc.gpsimd.collective_compute(
                "AllGather",
                mybir.AluOpType.bypass,
                replica_groups=[[2 * i, 2 * i + 1] for i in range(N_CORES // 2)],
                ins=[warm_b.ap().opt()],
                outs=[warm_g[:].opt()],
            )

            ones = constp.tile([P, P], fp32)
            nc.vector.memset(ones[:], 1.0)

            # ---- loads (w + hid first: they gate the v chain) ----
            h_sb = sb.tile([P, KCH], fp32)
            nc.sync.dma_start(h_sb[:], hidP.ap())
            w_sb = sb.tile([P, DSH * KCH], fp32)
            nc.sync.dma_start(w_sb[:], wP.ap())
            # enc split across two DMA queues (Sync + Scalar) to engage two
            # HWDGE rings; W rides first on Sync so only one ring competes
            # during its short transfer.
            enc_sb = sb.tile([P, F * DSH], fp32)
            for c in range(NCH):
                sl = slice(c * FC * DSH, (c + 1) * FC * DSH)
                eng = nc.sync if c < 2 else nc.scalar
                eng.dma_start(enc_sb[:, sl], encP.ap()[:, sl])

            # ---- v_i = W_i^T @ hidden (DVE mult + unit-stride reduce) ----
            prod_w = sb.tile([P, DSH * KCH], fp32)
            h_b = (
                h_sb[:]
                .rearrange("p k -> p () k")
                .broadcast_to([P, DSH, KCH])
            )
            nc.vector.tensor_tensor(
                out=prod_w[:].rearrange("p (d k) -> p d k", d=DSH),
                in0=w_sb[:].rearrange("p (d k) -> p d k", d=DSH),
                in1=h_b,
                op=mybir.AluOpType.mult,
            )
            vtmp = sb.tile([P, DSH], fp32)
            nc.vector.reduce_sum(
                vtmp[:],
                prod_w[:].rearrange("p (d k) -> p d k", d=DSH),
                axis=mybir.AxisListType.X,
            )
            # one matmul: column-sums broadcast to every partition
            v_bc_ps = ps.tile([P, DSH], fp32, tag="vbc")
            nc.tensor.matmul(
                v_bc_ps[:], lhsT=ones[:], rhs=vtmp[:], start=True, stop=True
            )
            v_bc = sb.tile([P, DSH], fp32)
            nc.scalar.copy(v_bc[:], v_bc_ps[:])

            # ---- partial energies e_i[p, f] = sum_d enc[p, f, d] * v[d];
            # each chunk's slice bounces to DRAM as soon as its reduce
            # lands so the last bounce overlaps the tail of the DVE ----
            bounce = dram.tile([P, F], fp32)
            esum = dram.tile([P, F], fp32, addr_space="Shared")
            e_part = sb.tile([P, F], fp32)
            for c in range(NCH):
                sl3 = enc_sb[:].rearrange("p (f d) -> p f d", d=DSH)[
                    :, c * FC : (c + 1) * FC, :
                ]
                eprod = sb.tile([P, FC * DSH], fp32, tag="eprod", bufs=2)
                v_b = (
                    v_bc[:]
                    .rearrange("p d -> p () d")
                    .broadcast_to([P, FC, DSH])
                )
                nc.vector.tensor_tensor(
                    out=eprod[:].rearrange("p (f d) -> p f d", d=DSH),
                    in0=sl3,
                    in1=v_b,
                    op=mybir.AluOpType.mult,
                )
                nc.vector.reduce_sum(
                    e_part[:, c * FC : (c + 1) * FC],
                    eprod[:].rearrange("p (f d) -> p f d", d=DSH),
                    axis=mybir.AxisListType.X,
                )
                nc.sync.dma_start(
                    bounce[:, c * FC : (c + 1) * FC],
                    e_part[:, c * FC : (c + 1) * FC],
                )
            nc.gpsimd.collective_compute(
                "AllReduce",
                mybir.AluOpType.add,
                replica_groups=rg,
                ins=[bounce[:].opt()],
                outs=[esum[:].opt()],
            )
            e_sb = sb.tile([P, F], fp32)
            nc.sync.dma_start(e_sb[:], esum[:])

            # ---- replicated softmax over [128, 256] via partition_all_reduce
            # (short serial chain, no PE transposes) ----
            m_p = sb.tile([P, 1], fp32)
            nc.vector.reduce_max(m_p[:], e_sb[:], axis=mybir.AxisListType.X)
            M_bc = sb.tile([P, 1], fp32)
            nc.gpsimd.partition_all_reduce(
                M_bc[:], m_p[:], channels=P, reduce_op=bass_isa.ReduceOp.max
            )
            negM = sb.tile([P, 1], fp32)
            nc.vector.tensor_scalar_mul(negM[:], M_bc[:], -1.0)
            q = sb.tile([P, F], fp32)
            s_p = sb.tile([P, 1], fp32)
            nc.scalar.activation(
                q[:], e_sb[:], mybir.ActivationFunctionType.Exp,
                bias=negM[:], scale=1.0, accum_out=s_p[:],
            )
            S_bc = sb.tile([P, 1], fp32)
            nc.gpsimd.partition_all_reduce(
                S_bc[:], s_p[:], channels=P, reduce_op=bass_isa.ReduceOp.add
            )
            rS = sb.tile([P, 1], fp32)
            nc.vector.reciprocal(rS[:], S_bc[:])
            o_sb = sb.tile([P, F], fp32)
            nc.vector.tensor_scalar_mul(o_sb[:], q[:], rS[:])
            nc.sync.dma_start(out.ap().rearrange("(p f) -> p f", p=P), o_sb[:])

            # Keep the warm-up collective live. tile_wait_until pins it to
            # the end of the Tile scheduler's timeline so its wait on the
            # warm AllGather never stalls real work queued after it.
            with tc.tile_wait_until(1.0):
                nc.scalar.dma_start(warm_out.ap(), warm_g[:])

    nc.compile()
    return nc


def shard_inputs(hidden, encoder_outputs, W, b):
    hidden = np.asarray(hidden, dtype=np.float32)
    enc = np.asarray(encoder_outputs, dtype=np.float32)
    W = np.asarray(W, dtype=np.float32)
    enc3 = enc.reshape(P, F, D)          # s = p*F + f
    w3 = W.reshape(KCH, P, D)            # h = k*P + p
    hidP = np.ascontiguousarray(hidden.reshape(KCH, P).T)  # [p, k]
    in_maps = []
    for i in range(N_CORES):
        sl = slice(i * DSH, (i + 1) * DSH)
        encP_i = np.ascontiguousarray(enc3[:, :, sl]).reshape(P, F * DSH)
        wP_i = np.ascontiguousarray(
            w3[:, :, sl].transpose(1, 2, 0)       # [p, d, k]
        ).reshape(P, DSH * KCH)
        in_maps.append({"encP": encP_i, "wP": wP_i, "hidP": hidP})
    return in_maps


_NC_CACHE = {}


def _get_nc():
    if "nc" not in _NC_CACHE:
        _NC_CACHE["nc"] = build_kernel()
    return _NC_CACHE["nc"]


def kernel(hidden, encoder_outputs, W, b):
    from concourse import bass_utils

    nc = _get_nc()
    in_maps = shard_inputs(hidden, encoder_outputs, W, b)
    res = bass_utils.run_bass_kernel_spmd(
        nc, in_maps, core_ids=list(range(N_CORES))
    )
    out = np.asarray(res.results[0]["out"], dtype=np.float32)
    return out.reshape(1, 1, SEQ)
